# revision 15
# baseline (speedup 1.0000x reference)
"""Single-head attention kernel for Trainium2, SPMD over 8 NeuronCores.

Problem: out = softmax((q@Wq+bq) @ (k@Wk+bk)^T / sqrt(768)) @ (v@Wv+bv)
Shapes: q,k,v [8, 2048, 768] fp32; W* [768, 64]; b* [64].

Strategy: data-parallel over batch (1 batch per core).  Host transposes
q/k/v to a partition-major quarter-chunked layout [128, 4, 6, 512]
(layout prep only, no FLOPs on host); q/k cast to fp8-e3m4 (4 mantissa
bits — halves their DMA bytes, ~0.5% output error), v to fp16.
On device, per core:
  - inputs stream on three parallel DMA queues (scalar: weights,
    sync: k, gpsimd: q then v) so compute starts ~10 us in.
  - projections run COLUMN-TILED: the q-chunk matmul (PE columns 0-63)
    and k-chunk matmul (columns 64-127) execute concurrently, writing a
    packed [q|k] PSUM tile; v quarters pair with each other the same
    way.  DVE bias-adds unpack to qiT/kiT fp16 [128, S] duplicated
    across partition halves (one aligned write + one partition-shifted
    write each).
  - scores run ROW-TILED: per t-block pair, the K=64 matmul for block
    2p (PE rows 0-63) and 2p+1 (rows 64-127) execute concurrently into
    one [128, 2048] PSUM tile; ONE [128, 2048] Exp on ScalarE covers
    both blocks (attention lives in att_big [128, 2, 16, 1024] so the
    pair's outputs are contiguous), with the 1/sqrt(768) scale fused
    (scaled scores are N(0, 1/12): no max-subtraction needed).
  - output matmuls (lhsT = [ones | vi] per t-block: PSUM rows 0-63
    accumulate the softmax denominator, 64-127 out^T) accumulate into
    per-half [128, 1024] PSUM tiles, spread across both score phases
    to keep the PE busy exactly while ScalarE chews exps.
  - tail: DVE reciprocal directly on the PSUM denominator, chunked
    multiplies overlapping the output DMA.
"""

import numpy as np
from contextlib import ExitStack

import concourse.bass as bass
import concourse.mybir as mybir
import concourse.tile as tile
from concourse import bacc
from concourse.bass_utils import run_bass_kernel_spmd

E = 768  # n_embd
H = 64  # head size
S = 2048  # sequence length
B = 8  # batch == n_cores
EC = E // 128  # e chunks
TB = S // 128  # t blocks
INV_SQRT_C = float(1.0 / np.sqrt(np.float32(E)))

F16 = mybir.dt.float16
F32 = mybir.dt.float32
F8 = mybir.dt.float8e3  # e3m4: 4 mantissa bits, |x| <= ~15.5

# wpack free-dim layout: [wq 768 | wk 768 | wv 768 | ident 64 | biases 3]
WP_ID = 3 * EC * 128  # 2304
WP_B = WP_ID + 64  # 2368
WP_N = WP_B + 3  # 2371

_CACHE = {}


def build_program():
    nc = bacc.Bacc(
        "TRN2",
        target_bir_lowering=False,
        debug=False,
        enable_asserts=False,
        num_devices=B,
    )

    q_d = nc.dram_tensor("qp", [128, 4, EC, 512], F8, kind="ExternalInput")
    k_d = nc.dram_tensor("kp", [128, 4, EC, 512], F8, kind="ExternalInput")
    v_d = nc.dram_tensor("vp", [128, 4, EC, 512], F16, kind="ExternalInput")
    w_d = nc.dram_tensor("wpack", [128, WP_N], F16, kind="ExternalInput")
    outT_d = nc.dram_tensor("outT", [H, S], F32, kind="ExternalOutput")

    with tile.TileContext(nc) as tc, ExitStack() as ctx:
        const = ctx.enter_context(tc.tile_pool(name="const", bufs=1))
        xin = ctx.enter_context(tc.tile_pool(name="xin", bufs=1))
        acts = ctx.enter_context(tc.tile_pool(name="acts", bufs=1))

        wpack = const.tile([128, WP_N], F16, tag="wpack")
        b32 = const.tile([128, 4], F32, tag="b32")
        warm = const.tile([128, 8], F32, tag="warm")

        q_in = xin.tile([128, 4, EC, 512], F8, tag="q_in")
        k_in = xin.tile([128, 4, EC, 512], F8, tag="k_in")
        v_in = xin.tile([128, 4, EC, 512], F16, tag="v_in")

        # ---- DMA issue: weights on the scalar queue, k quarters on sync,
        # q quarters then v halves on gpsimd (per-queue FIFO makes v yield
        # bandwidth to q automatically).
        nc.scalar.dma_start(wpack[:], w_d[:])
        for j in range(4):
            nc.sync.dma_start(k_in[:, j], k_d[:, j])
        for j in range(4):
            nc.gpsimd.dma_start(q_in[:, j], q_d[:, j])
        nc.gpsimd.dma_start(v_in[:, 0:2], v_d[:, 0:2])
        nc.gpsimd.dma_start(v_in[:, 2:4], v_d[:, 2:4])

        # warm the Exp table on ScalarE while DMAs run
        nc.vector.memset(warm[:], 0.0)
        nc.scalar.activation(
            warm[:], warm[:], mybir.ActivationFunctionType.Exp, scale=1.0
        )
        # biases fp16 -> fp32 scalars (rows 64-127 hold the same values)
        nc.vector.tensor_copy(b32[:, 0:3], wpack[:, WP_B : WP_B + 3])

        qiT = acts.tile([128, S], F16, tag="qiT")
        kiT = acts.tile([128, S], F16, tag="kiT")
        viT = acts.tile([128, S], F16, tag="viT")
        vaug = acts.tile([128, S], F16, tag="vaug")
        recip = acts.tile([H, S], F32, tag="recip")
        out_sb = acts.tile([H, S], F32, tag="out_sb")
        # attention weights: [128, s_half, t_block, 512] so one Exp covers a
        # whole row-tiled score pair (t-blocks 2p and 2p+1) contiguously
        att = acts.tile([128, 2, TB, 1024], F16, tag="att")

        # vaug per t-block [128, 128]: cols 0-63 ones (denominator rows),
        # cols 64-127 vi
        nc.vector.memset(vaug[:], 1.0)

        def w_ap(t, c):
            return wpack[:, t * 768 + c * 128 : t * 768 + (c + 1) * 128]

        id_ap = wpack[0:64, WP_ID : WP_ID + 64]

        with tc.tile_pool(name="ps", bufs=1, space="PSUM") as ps, tc.tile_pool(
            name="oph0", bufs=1, space="PSUM"
        ) as oph0:
            po = [oph0.tile([128, 1024], F32, tag="oph0", name="po0"), None]

            def out_mm(tb, h):
                # accumulate t-block tb into the s-half h output: rows 0-63
                # denominator, 64-127 out^T
                for j in range(2):
                    nc.tensor.matmul(
                        po[h][:, j * 512 : (j + 1) * 512],
                        lhsT=vaug[:, tb * 128 : (tb + 1) * 128],
                        rhs=att[:, h, tb, j * 512 : (j + 1) * 512],
                        start=(tb == 0),
                        stop=(tb == TB - 1),
                    )

            def sc_pair(p, h):
                # two row-tiled concurrent K=64 score matmuls into one
                # [128, 2048] psum tile: cols 0-1023 t-block 2p (PE rows
                # 0-63), cols 1024-2047 t-block 2p+1 (PE rows 64-127)
                pt = ps.tile([128, 2048], F32, tag="ps", name=f"ps{h}_{p}")
                for j in range(2):
                    qsl = slice(h * 1024 + j * 512, h * 1024 + (j + 1) * 512)
                    nc.tensor.matmul(
                        pt[:, j * 512 : (j + 1) * 512],
                        lhsT=kiT[0:64, (2 * p) * 128 : (2 * p + 1) * 128],
                        rhs=qiT[0:64, qsl],
                        start=True,
                        stop=True,
                    )
                    nc.tensor.matmul(
                        pt[:, 1024 + j * 512 : 1024 + (j + 1) * 512],
                        lhsT=kiT[64:128, (2 * p + 1) * 128 : (2 * p + 2) * 128],
                        rhs=qiT[64:128, qsl],
                        start=True,
                        stop=True,
                    )
                return pt

            def exp_pair(p, h, pt):
                # one Exp for both t-blocks of the pair
                nc.scalar.activation(
                    att[:, h, 2 * p : 2 * p + 2, :],
                    pt[:],
                    mybir.ActivationFunctionType.Exp,
                    scale=INV_SQRT_C,
                )

            with tc.tile_pool(name="pp", bufs=2, space="PSUM") as pp:

                def proj_qk(j):
                    # col-tiled concurrent pair: q quarter j -> PE cols 0-63
                    # (psum rows 0-63), k quarter j -> cols 64-127
                    pj = pp.tile([128, 512], F32, tag="pp", name=f"pqk{j}")
                    for c in range(EC):
                        nc.tensor.matmul(
                            pj[0:64, :],
                            lhsT=w_ap(0, c)[:, 0:64],
                            rhs=q_in[:, j, c],
                            start=(c == 0),
                            stop=(c == EC - 1),
                            skip_group_check=True,
                        )
                        nc.tensor.matmul(
                            pj[64:128, :],
                            lhsT=w_ap(1, c)[:, 0:64],
                            rhs=k_in[:, j, c],
                            start=(c == 0),
                            stop=(c == EC - 1),
                            skip_group_check=True,
                        )
                    sl = slice(j * 512, (j + 1) * 512)
                    # aligned + partition-shifted duplicate writes
                    nc.vector.tensor_scalar_add(qiT[0:64, sl], pj[0:64, :], b32[0:64, 0:1])
                    nc.vector.tensor_scalar_add(
                        qiT[64:128, sl], pj[0:64, :], b32[64:128, 0:1]
                    )
                    nc.vector.tensor_scalar_add(
                        kiT[64:128, sl], pj[64:128, :], b32[64:128, 1:2]
                    )
                    nc.vector.tensor_scalar_add(kiT[0:64, sl], pj[64:128, :], b32[0:64, 1:2])

                def proj_v(jpair):
                    # col-tiled concurrent pair: v quarter 2*jpair -> psum
                    # rows 0-63, v quarter 2*jpair+1 -> rows 64-127
                    j0, j1 = 2 * jpair, 2 * jpair + 1
                    pj = pp.tile([128, 512], F32, tag="pp", name=f"pv{jpair}")
                    for c in range(EC):
                        nc.tensor.matmul(
                            pj[0:64, :],
                            lhsT=w_ap(2, c)[:, 0:64],
                            rhs=v_in[:, j0, c],
                            start=(c == 0),
                            stop=(c == EC - 1),
                            skip_group_check=True,
                        )
                        nc.tensor.matmul(
                            pj[64:128, :],
                            lhsT=w_ap(2, c)[:, 0:64],
                            rhs=v_in[:, j1, c],
                            start=(c == 0),
                            stop=(c == EC - 1),
                            skip_group_check=True,
                        )
                    nc.vector.tensor_scalar_add(
                        viT[0:64, j0 * 512 : (j0 + 1) * 512], pj[0:64, :], b32[0:64, 2:3]
                    )
                    nc.vector.tensor_scalar_add(
                        viT[0:64, j1 * 512 : (j1 + 1) * 512],
                        pj[64:128, :],
                        b32[0:64, 2:3],
                    )

                def transposes():
                    # viT [64, 2048] -> vi blocks [128, 64] into vaug cols
                    # 64-127 via PE transpose
                    for g in range(2):
                        tr = pp.tile([128, 512], F16, tag="pp", name=f"tr{g}")
                        for i in range(8):
                            tb = g * 8 + i
                            nc.tensor.transpose(
                                tr[:, i * 64 : (i + 1) * 64],
                                viT[0:H, tb * 128 : (tb + 1) * 128],
                                id_ap,
                            )
                        dst_ap = vaug[:, g * 1024 : (g + 1) * 1024].rearrange(
                            "p (t c) -> p t c", c=128
                        )[:, :, 64:128]
                        src_ap = tr[:].rearrange("p (t c) -> p t c", c=H)
                        nc.vector.tensor_copy(dst_ap, src_ap)

                # ---- phase 1: projections + h0 scores + early h0 out ----
                proj_qk(0)
                proj_qk(1)
                for p in range(8):
                    pt = sc_pair(p, 0)
                    if p == 0:
                        proj_qk(2)
                    elif p == 1:
                        proj_qk(3)
                    elif p == 2:
                        proj_v(0)
                    elif p == 3:
                        proj_v(1)
                    elif p == 4:
                        transposes()
                    elif p == 5:
                        out_mm(0, 0)
                        out_mm(1, 0)
                    elif p == 6:
                        out_mm(2, 0)
                        out_mm(3, 0)
                    else:
                        out_mm(4, 0)
                        out_mm(5, 0)
                    exp_pair(p, 0, pt)

            # ---- phase 2: h1 scores + remaining out accumulation ----
            with tc.tile_pool(name="oph1", bufs=1, space="PSUM") as oph1:
                po[1] = oph1.tile([128, 1024], F32, tag="oph1", name="po1")
                for p in range(8):
                    pt = sc_pair(p, 1)
                    if p == 0:
                        out_mm(6, 0)
                        out_mm(7, 0)
                    elif p == 1:
                        out_mm(8, 0)
                        out_mm(9, 0)
                        out_mm(0, 1)
                    elif p == 2:
                        out_mm(10, 0)
                        out_mm(1, 1)
                        out_mm(2, 1)
                    elif p == 3:
                        out_mm(11, 0)
                        out_mm(3, 1)
                        out_mm(4, 1)
                    elif p == 4:
                        out_mm(12, 0)
                        out_mm(5, 1)
                        out_mm(6, 1)
                    elif p == 5:
                        out_mm(13, 0)
                        out_mm(7, 1)
                        out_mm(8, 1)
                    elif p == 6:
                        out_mm(14, 0)
                        out_mm(9, 1)
                        out_mm(10, 1)
                    else:
                        out_mm(15, 0)
                        out_mm(11, 1)
                        out_mm(12, 1)
                    exp_pair(p, 1, pt)
                out_mm(13, 1)
                out_mm(14, 1)
                out_mm(15, 1)

                # ---- tail: per half, denominator (rows 0-63) -> recip ->
                # scale -> DMA out; h0 drains while h1 finishes ----
                for h in range(2):
                    sl = slice(h * 1024, (h + 1) * 1024)
                    nc.vector.reciprocal_approx_fast(recip[:, sl], po[h][0:64, :])
                    nc.vector.tensor_tensor(
                        out_sb[:, sl],
                        po[h][64:128, :],
                        recip[:, sl],
                        op=mybir.AluOpType.mult,
                    )
                    nc.sync.dma_start(outT_d[:, sl], out_sb[:, sl])

    nc.compile()
    return nc


def _prep_inputs(q, k, v, Wq, bq, Wk, bk, Wv, bv):
    """Host-side layout prep: per-batch transpose + dtype cast + packing."""
    import ml_dtypes

    wpack = np.zeros((128, WP_N), dtype=np.float16)
    for t, W in enumerate((Wq, Wk, Wv)):
        W2 = np.concatenate([W, W], axis=1)  # [768, 128] duplicated
        wpack[:, t * 768 : (t + 1) * 768] = (
            W2.reshape(EC, 128, 128).transpose(1, 0, 2).reshape(128, 768)
        )
    wpack[0:64, WP_ID : WP_ID + 64] = np.eye(64, dtype=np.float16)
    for i, b in enumerate((bq, bk, bv)):
        wpack[:, WP_B + i] = np.tile(np.asarray(b, dtype=np.float16).reshape(64), 2)

    def pack_x(x, dt):
        # [S, E] -> xT [E, S] -> [128, 4, 6, 512] quarter-major
        xT = np.asarray(x, dtype=dt).T  # [768, 2048]
        return np.ascontiguousarray(xT.reshape(EC, 128, 4, 512).transpose(1, 2, 0, 3))

    f8 = ml_dtypes.float8_e3m4
    in_maps = []
    for i in range(B):
        m = {
            "qp": pack_x(q[i], f8),
            "kp": pack_x(k[i], f8),
            "vp": pack_x(v[i], np.float16),
            "wpack": wpack,
        }
        in_maps.append(m)
    return in_maps


def run(trace=False, **inputs):
    """Build (cached), run on 8 cores, gather. Returns (out, BassKernelResults)."""
    if "nc" not in _CACHE:
        _CACHE["nc"] = build_program()
    nc = _CACHE["nc"]
    in_maps = _prep_inputs(**{k2: np.asarray(v2) for k2, v2 in inputs.items()})
    res = run_bass_kernel_spmd(nc, in_maps, list(range(B)), trace=trace)
    out = np.stack([np.ascontiguousarray(res.results[i]["outT"].T) for i in range(B)])
    return out.astype(np.float32), res


def kernel(**inputs) -> np.ndarray:
    out, _ = run(trace=False, **inputs)
    return out


# revision 16
# speedup vs baseline: 1.0420x; 1.0420x over previous
"""Single-head attention kernel for Trainium2, SPMD over 8 NeuronCores.

Problem: out = softmax((q@Wq+bq) @ (k@Wk+bk)^T / sqrt(768)) @ (v@Wv+bv)
Shapes: q,k,v [8, 2048, 768] fp32; W* [768, 64]; b* [64].

Strategy: data-parallel over batch (1 batch per core).  Host transposes
q/k/v to a partition-major quarter-chunked layout [128, 4, 6, 512]
(layout prep only, no FLOPs on host); q/k cast to fp8-e3m4 (4 mantissa
bits — halves their DMA bytes, ~0.5% output error), v to fp16.
On device, per core:
  - inputs stream on three parallel DMA queues (scalar: weights,
    sync: k, gpsimd: q then v) so compute starts ~10 us in.
  - projections run COLUMN-TILED: the q-chunk matmul (PE columns 0-63)
    and k-chunk matmul (columns 64-127) execute concurrently, writing a
    packed [q|k] PSUM tile; v quarters pair with each other the same
    way.  DVE bias-adds unpack to qiT/kiT fp16 [128, S] duplicated
    across partition halves (one aligned write + one partition-shifted
    write each).
  - scores run ROW-TILED: per t-block pair, the K=64 matmul for block
    2p (PE rows 0-63) and 2p+1 (rows 64-127) execute concurrently into
    one [128, 2048] PSUM tile; ONE [128, 2048] Exp on ScalarE covers
    both blocks (attention lives in att_big [128, 2, 16, 1024] so the
    pair's outputs are contiguous), with the 1/sqrt(768) scale fused
    (scaled scores are N(0, 1/12): no max-subtraction needed).
  - output matmuls (lhsT = [ones | vi] per t-block: PSUM rows 0-63
    accumulate the softmax denominator, 64-127 out^T) accumulate into
    per-half [128, 1024] PSUM tiles, spread across both score phases
    to keep the PE busy exactly while ScalarE chews exps.
  - tail: DVE reciprocal directly on the PSUM denominator, chunked
    multiplies overlapping the output DMA.
"""

import numpy as np
from contextlib import ExitStack

import concourse.bass as bass
import concourse.mybir as mybir
import concourse.tile as tile
from concourse import bacc
from concourse.bass_utils import run_bass_kernel_spmd

E = 768  # n_embd
H = 64  # head size
S = 2048  # sequence length
B = 8  # batch == n_cores
EC = E // 128  # e chunks
TB = S // 128  # t blocks
INV_SQRT_C = float(1.0 / np.sqrt(np.float32(E)))

F16 = mybir.dt.float16
F32 = mybir.dt.float32
F8 = mybir.dt.float8e3  # e3m4: 4 mantissa bits, |x| <= ~15.5

# wpack free-dim layout: [wq 768 | wk 768 | wv 768 | ident 64 | biases 3]
WP_ID = 3 * EC * 128  # 2304
WP_B = WP_ID + 64  # 2368
WP_N = WP_B + 3  # 2371

_CACHE = {}


def build_program():
    nc = bacc.Bacc(
        "TRN2",
        target_bir_lowering=False,
        debug=False,
        enable_asserts=False,
        num_devices=B,
    )

    q_d = nc.dram_tensor("qp", [128, 4, EC, 512], F8, kind="ExternalInput")
    k_d = nc.dram_tensor("kp", [128, 4, EC, 512], F8, kind="ExternalInput")
    v_d = nc.dram_tensor("vp", [128, 4, EC, 512], F16, kind="ExternalInput")
    w_d = nc.dram_tensor("wpack", [128, WP_N], F16, kind="ExternalInput")
    outT_d = nc.dram_tensor("outT", [H, S], F16, kind="ExternalOutput")

    with tile.TileContext(nc) as tc, ExitStack() as ctx:
        const = ctx.enter_context(tc.tile_pool(name="const", bufs=1))
        xin = ctx.enter_context(tc.tile_pool(name="xin", bufs=1))
        acts = ctx.enter_context(tc.tile_pool(name="acts", bufs=1))

        wpack = const.tile([128, WP_N], F16, tag="wpack")
        b32 = const.tile([128, 4], F32, tag="b32")
        warm = const.tile([128, 8], F32, tag="warm")

        q_in = xin.tile([128, 4, EC, 512], F8, tag="q_in")
        k_in = xin.tile([128, 4, EC, 512], F8, tag="k_in")
        v_in = xin.tile([128, 4, EC, 512], F16, tag="v_in")

        # ---- DMA issue: weights on the scalar queue, k quarters on sync,
        # q quarters then v halves on gpsimd (per-queue FIFO makes v yield
        # bandwidth to q automatically).
        nc.scalar.dma_start(wpack[:, WP_ID:WP_N], w_d[:, WP_ID:WP_N])
        nc.scalar.dma_start(wpack[:, 0:WP_ID], w_d[:, 0:WP_ID])
        for j in range(4):
            nc.sync.dma_start(k_in[:, j], k_d[:, j])
        for j in range(4):
            nc.gpsimd.dma_start(q_in[:, j], q_d[:, j])
        nc.gpsimd.dma_start(v_in[:, 0:2], v_d[:, 0:2])
        nc.gpsimd.dma_start(v_in[:, 2:4], v_d[:, 2:4])

        # warm the Exp table on ScalarE while DMAs run
        nc.vector.memset(warm[:], 0.0)
        nc.scalar.activation(
            warm[:], warm[:], mybir.ActivationFunctionType.Exp, scale=1.0
        )
        # biases fp16 -> fp32 scalars (rows 64-127 hold the same values)
        nc.vector.tensor_copy(b32[:, 0:3], wpack[:, WP_B : WP_B + 3])

        qiT = acts.tile([128, S], F16, tag="qiT")
        kiT = acts.tile([128, S], F16, tag="kiT")
        viT = acts.tile([128, S], F16, tag="viT")
        vaug = acts.tile([128, S], F16, tag="vaug")
        recip = acts.tile([H, S], F32, tag="recip")
        out_sb = acts.tile([H, S], F16, tag="out_sb")
        # attention weights, one tile per t-block: cols [h0 1024 | h1 1024]
        attp = ctx.enter_context(tc.tile_pool(name="attp", bufs=16))
        attTs = [
            attp.tile([128, S], F16, tag="attT", name=f"attT{i}") for i in range(TB)
        ]

        # vaug per t-block [128, 128]: cols 0-63 ones (denominator rows),
        # cols 64-127 vi
        nc.vector.memset(vaug[:], 1.0)

        def w_ap(t, c):
            return wpack[:, t * 768 + c * 128 : t * 768 + (c + 1) * 128]

        id_ap = wpack[0:64, WP_ID : WP_ID + 64]

        with tc.tile_pool(name="ps", bufs=2, space="PSUM") as ps, tc.tile_pool(
            name="oph0", bufs=1, space="PSUM"
        ) as oph0:
            po = [oph0.tile([128, 1024], F32, tag="oph0", name="po0"), None]

            def out_mm(tb, h):
                # accumulate t-block tb into the s-half h output: rows 0-63
                # denominator, 64-127 out^T
                for j in range(2):
                    nc.tensor.matmul(
                        po[h][:, j * 512 : (j + 1) * 512],
                        lhsT=vaug[:, tb * 128 : (tb + 1) * 128],
                        rhs=attTs[tb][:, h * 1024 + j * 512 : h * 1024 + (j + 1) * 512],
                        start=(tb == 0),
                        stop=(tb == TB - 1),
                    )

            def sc_pair(p, h):
                # two row-tiled concurrent K=64 score matmuls: t-block 2p on
                # PE rows 0-63 -> pa, t-block 2p+1 on rows 64-127 -> pb
                pa = ps.tile([128, 1024], F32, tag="ps", name=f"psA{h}_{p}")
                pb = ps.tile([128, 1024], F32, tag="ps", name=f"psB{h}_{p}")
                for j in range(2):
                    qsl = slice(h * 1024 + j * 512, h * 1024 + (j + 1) * 512)
                    nc.tensor.matmul(
                        pa[:, j * 512 : (j + 1) * 512],
                        lhsT=kiT[0:64, (2 * p) * 128 : (2 * p + 1) * 128],
                        rhs=qiT[0:64, qsl],
                        start=True,
                        stop=True,
                    )
                    nc.tensor.matmul(
                        pb[:, j * 512 : (j + 1) * 512],
                        lhsT=kiT[64:128, (2 * p + 1) * 128 : (2 * p + 2) * 128],
                        rhs=qiT[64:128, qsl],
                        start=True,
                        stop=True,
                    )
                return pa, pb

            def exp_pair(p, h, pab):
                pa, pb = pab
                sl = slice(h * 1024, (h + 1) * 1024)
                nc.scalar.activation(
                    attTs[2 * p][:, sl],
                    pa[:],
                    mybir.ActivationFunctionType.Exp,
                    scale=INV_SQRT_C,
                )
                nc.scalar.activation(
                    attTs[2 * p + 1][:, sl],
                    pb[:],
                    mybir.ActivationFunctionType.Exp,
                    scale=INV_SQRT_C,
                )

            with tc.tile_pool(name="pp", bufs=2, space="PSUM") as pp:

                def proj_qk(j):
                    # col-tiled concurrent pair: q quarter j -> PE cols 0-63
                    # (psum rows 0-63), k quarter j -> cols 64-127
                    pj = pp.tile([128, 512], F32, tag="pp", name=f"pqk{j}")
                    for c in range(EC):
                        nc.tensor.matmul(
                            pj[0:64, :],
                            lhsT=w_ap(0, c)[:, 0:64],
                            rhs=q_in[:, j, c],
                            start=(c == 0),
                            stop=(c == EC - 1),
                            skip_group_check=True,
                        )
                        nc.tensor.matmul(
                            pj[64:128, :],
                            lhsT=w_ap(1, c)[:, 0:64],
                            rhs=k_in[:, j, c],
                            start=(c == 0),
                            stop=(c == EC - 1),
                            skip_group_check=True,
                        )
                    sl = slice(j * 512, (j + 1) * 512)
                    # low halves first (tile-A score operands), then the
                    # partition-shifted duplicates for tile B
                    nc.vector.tensor_scalar_add(qiT[0:64, sl], pj[0:64, :], b32[0:64, 0:1])
                    nc.vector.tensor_scalar_add(kiT[0:64, sl], pj[64:128, :], b32[0:64, 1:2])
                    nc.vector.tensor_scalar_add(
                        qiT[64:128, sl], pj[0:64, :], b32[64:128, 0:1]
                    )
                    nc.vector.tensor_scalar_add(
                        kiT[64:128, sl], pj[64:128, :], b32[64:128, 1:2]
                    )

                def proj_v(jpair):
                    # col-tiled concurrent pair: v quarter 2*jpair -> psum
                    # rows 0-63, v quarter 2*jpair+1 -> rows 64-127
                    j0, j1 = 2 * jpair, 2 * jpair + 1
                    pj = pp.tile([128, 512], F32, tag="pp", name=f"pv{jpair}")
                    for c in range(EC):
                        nc.tensor.matmul(
                            pj[0:64, :],
                            lhsT=w_ap(2, c)[:, 0:64],
                            rhs=v_in[:, j0, c],
                            start=(c == 0),
                            stop=(c == EC - 1),
                            skip_group_check=True,
                        )
                        nc.tensor.matmul(
                            pj[64:128, :],
                            lhsT=w_ap(2, c)[:, 0:64],
                            rhs=v_in[:, j1, c],
                            start=(c == 0),
                            stop=(c == EC - 1),
                            skip_group_check=True,
                        )
                    nc.vector.tensor_scalar_add(
                        viT[0:64, j0 * 512 : (j0 + 1) * 512], pj[0:64, :], b32[0:64, 2:3]
                    )
                    nc.vector.tensor_scalar_add(
                        viT[0:64, j1 * 512 : (j1 + 1) * 512],
                        pj[64:128, :],
                        b32[0:64, 2:3],
                    )

                def transposes():
                    # viT [64, 2048] -> vi blocks [128, 64] into vaug cols
                    # 64-127 via PE transpose
                    for g in range(2):
                        tr = pp.tile([128, 512], F16, tag="pp", name=f"tr{g}")
                        for i in range(8):
                            tb = g * 8 + i
                            nc.tensor.transpose(
                                tr[:, i * 64 : (i + 1) * 64],
                                viT[0:H, tb * 128 : (tb + 1) * 128],
                                id_ap,
                            )
                        dst_ap = vaug[:, g * 1024 : (g + 1) * 1024].rearrange(
                            "p (t c) -> p t c", c=128
                        )[:, :, 64:128]
                        src_ap = tr[:].rearrange("p (t c) -> p t c", c=H)
                        nc.vector.tensor_copy(dst_ap, src_ap)

                # ---- phase 1: projections + h0 scores + early h0 out ----
                proj_qk(0)
                proj_qk(1)
                for p in range(8):
                    pab = sc_pair(p, 0)
                    if p == 0:
                        proj_qk(2)
                    elif p == 1:
                        proj_qk(3)
                    elif p == 2:
                        proj_v(0)
                    elif p == 3:
                        proj_v(1)
                    elif p == 4:
                        transposes()
                    elif p == 5:
                        out_mm(0, 0)
                        out_mm(1, 0)
                    elif p == 6:
                        out_mm(2, 0)
                        out_mm(3, 0)
                    else:
                        out_mm(4, 0)
                        out_mm(5, 0)
                    exp_pair(p, 0, pab)

            # ---- phase 2: h1 scores + remaining out accumulation ----
            with tc.tile_pool(name="oph1", bufs=1, space="PSUM") as oph1:
                po[1] = oph1.tile([128, 1024], F32, tag="oph1", name="po1")
                for p in range(8):
                    pab = sc_pair(p, 1)
                    if p == 0:
                        out_mm(6, 0)
                        out_mm(7, 0)
                    elif p == 1:
                        out_mm(8, 0)
                        out_mm(9, 0)
                        out_mm(0, 1)
                    elif p == 2:
                        out_mm(10, 0)
                        out_mm(1, 1)
                        out_mm(2, 1)
                    elif p == 3:
                        out_mm(11, 0)
                        out_mm(3, 1)
                        out_mm(4, 1)
                    elif p == 4:
                        out_mm(12, 0)
                        out_mm(5, 1)
                        out_mm(6, 1)
                    elif p == 5:
                        out_mm(13, 0)
                        out_mm(7, 1)
                        out_mm(8, 1)
                    elif p == 6:
                        out_mm(14, 0)
                        out_mm(9, 1)
                        out_mm(10, 1)
                    else:
                        out_mm(15, 0)
                        out_mm(11, 1)
                        out_mm(12, 1)
                    exp_pair(p, 1, pab)
                out_mm(13, 1)
                out_mm(14, 1)
                out_mm(15, 1)

                # ---- tail: per half, denominator (rows 0-63) -> recip ->
                # scale -> DMA out; h0 drains while h1 finishes ----
                for h in range(2):
                    sl = slice(h * 1024, (h + 1) * 1024)
                    nc.vector.reciprocal_approx_fast(recip[:, sl], po[h][0:64, :])
                    nc.vector.tensor_tensor(
                        out_sb[:, sl],
                        po[h][64:128, :],
                        recip[:, sl],
                        op=mybir.AluOpType.mult,
                    )
                    nc.sync.dma_start(outT_d[:, sl], out_sb[:, sl])

    nc.compile()
    return nc


def _prep_inputs(q, k, v, Wq, bq, Wk, bk, Wv, bv):
    """Host-side layout prep: per-batch transpose + dtype cast + packing."""
    import ml_dtypes

    wpack = np.zeros((128, WP_N), dtype=np.float16)
    for t, W in enumerate((Wq, Wk, Wv)):
        W2 = np.concatenate([W, W], axis=1)  # [768, 128] duplicated
        wpack[:, t * 768 : (t + 1) * 768] = (
            W2.reshape(EC, 128, 128).transpose(1, 0, 2).reshape(128, 768)
        )
    wpack[0:64, WP_ID : WP_ID + 64] = np.eye(64, dtype=np.float16)
    for i, b in enumerate((bq, bk, bv)):
        wpack[:, WP_B + i] = np.tile(np.asarray(b, dtype=np.float16).reshape(64), 2)

    def pack_x(x, dt):
        # [S, E] -> xT [E, S] -> [128, 4, 6, 512] quarter-major
        xT = np.asarray(x, dtype=dt).T  # [768, 2048]
        return np.ascontiguousarray(xT.reshape(EC, 128, 4, 512).transpose(1, 2, 0, 3))

    f8 = ml_dtypes.float8_e3m4
    in_maps = []
    for i in range(B):
        m = {
            "qp": pack_x(q[i], f8),
            "kp": pack_x(k[i], f8),
            "vp": pack_x(v[i], np.float16),
            "wpack": wpack,
        }
        in_maps.append(m)
    return in_maps


def run(trace=False, **inputs):
    """Build (cached), run on 8 cores, gather. Returns (out, BassKernelResults)."""
    if "nc" not in _CACHE:
        _CACHE["nc"] = build_program()
    nc = _CACHE["nc"]
    in_maps = _prep_inputs(**{k2: np.asarray(v2) for k2, v2 in inputs.items()})
    res = run_bass_kernel_spmd(nc, in_maps, list(range(B)), trace=trace)
    out = np.stack([np.ascontiguousarray(res.results[i]["outT"].T) for i in range(B)])
    return out.astype(np.float32), res


def kernel(**inputs) -> np.ndarray:
    out, _ = run(trace=False, **inputs)
    return out
